# revision 9
# baseline (speedup 1.0000x reference)
"""Trainium2 Bass kernel for a graph decoder block (gnn_message_passing).

Contract: kernel(**inputs) takes the FULL unsharded inputs
(mol_annot [16,128,128], prot_annot [16,128,128], mol_adj [16,128,128,128],
prot_adj [16,128,128,128], params dict) and returns the full 4-tuple of
outputs, matching reference._decoder_block.

Strategy:
  - Data-parallel over batch: 16 samples -> 8 NeuronCores x 2 samples.
  - Per sample the heavy work is over the two edge tensors (128 i-tiles of
    [j=128, ch=128]).  Edge tiles are processed in "supertiles" of G=4
    i-tiles so vector-engine ops run at [128, 512] granularity.
  - Residual spine stays in natural (A) layout fp32; matmul/softmax interior
    runs transposed (C layout: ch on partitions) in bf16.  PE transposes
    (via identity matmul) convert between the two.
  - LayerNorm stats via bn_stats (even/odd split) + batched per-supertile
    scalar math; rstd computed as exp(-0.5*ln(var+eps)) so the single ACT
    table "natural_log_exp_and_others" covers every activation used.
  - Attention gating uses scalar_tensor_tensor chains:
        kq  = kT * q_i          (broadcast, bf16 2x)
        t   = (e_psum + be) * kq
        s   = (e_psum + be+1) * t
        p   = exp(s);  den = reduce_j(p);  num = reduce_j(p*vT)
"""
import math
from contextlib import ExitStack

import numpy as np
import ml_dtypes

import bass_rust
import concourse.bass as bass
import concourse.tile as tile
from concourse import mybir
from concourse.vector_clock import ScopedClock

F32 = mybir.dt.float32
BF16 = mybir.dt.bfloat16
AX = mybir.AxisListType
ALU = mybir.AluOpType
ACTF = mybir.ActivationFunctionType
ts = bass.ts

N_CORES = 8
B, N, C = 16, 128, 128
BPC = B // N_CORES          # samples per core
G = 4                       # i-tiles per supertile
NSUP = N // G               # supertiles per edge tensor
EPS = 1e-5

# weight pack order (each [128,128], stored as lhsT = W i.e. [ch_in, ch_out])
_WNAMES = [
    "attn2.q", "attn2.k", "attn2.v", "attn2.e", "attn2.out_e", "attn2.out_n",
    "dec.q_mx", "dec.k_px", "dec.v_mx", "dec.v_ma", "dec.k_pa",
    "dec.out_nd", "dec.out_ed",
    "mlp_ma.fc1", "mlp_ma.fc2", "mlp_mx.fc1", "mlp_mx.fc2",
]
_WIDX = {n: i for i, n in enumerate(_WNAMES)}
# bias col pack order ([128,1] each); 'attn2.e+1' is (be + 1)
_BNAMES = [
    "attn2.q", "attn2.k", "attn2.v", "attn2.e", "attn2.e+1", "attn2.out_e",
    "attn2.out_n", "dec.q_mx", "dec.k_px", "dec.v_mx", "dec.v_ma",
    "dec.k_pa+1", "dec.out_nd", "dec.out_ed",
    "mlp_ma.fc1", "mlp_ma.fc2", "mlp_mx.fc1", "mlp_mx.fc2", "eps",
]
_BIDX = {n: i for i, n in enumerate(_BNAMES)}
_LNS = ["ln1_ma", "ln1_pa", "ln1_mx", "ln1_px", "ln2_pa", "ln2_px",
        "ln3_ma", "ln3_mx", "ln4_ma", "ln4_mx"]


class _TC(tile.TileContext):
    """TileContext that legalizes sem-wait counts for this walrus.

    The walrus in this toolchain rejects instructions carrying more
    semaphore waits than their ISA struct encodes ("Too many sync wait
    commands").  Tile's sem assignment freely attaches 3-5 waits, so:
      - excess waits on scheduled instructions are split onto fresh NoOps
        on the same engine, inserted just before the instruction;
      - the tail drain's waits are likewise split one-per-instruction.
    """

    WAIT_CAP_DEFAULT = 1
    WAIT_CAPS = {"InstDrain": 1, "InstNoOp": 1, "InstEventSemaphore": 2}

    def _split_excess_waits(self, ordered):
        nsplit = 0
        for bb_name, insts in ordered.items():
            out = []
            for inst in insts:
                si = inst.sync_info
                waits = list(si.on_wait) if si else []
                cap = self.WAIT_CAPS.get(type(inst).__name__,
                                         self.WAIT_CAP_DEFAULT)
                if len(waits) > cap:
                    k = len(waits) - cap
                    for i in range(k):
                        n = mybir.InstNoOp(
                            name=f"{inst.name}-xw{i}")
                        n.engine = inst.engine
                        n.bass_nofuse = True
                        if inst.debug is not None:
                            n.debug = inst.debug
                        n.sync_info = bass_rust.SyncInfo(
                            on_wait=[waits[i]], on_update=[])
                        self.nc.register_instruction(n, overwrite=True)
                        out.append(n)
                        nsplit += 1
                    inst.sync_info = bass_rust.SyncInfo(
                        on_wait=waits[k:], on_update=list(si.on_update))
                out.append(inst)
            ordered[bb_name] = out

    def _lower_ordered_insts(self, ordered):
        self._split_excess_waits(ordered)
        return super()._lower_ordered_insts(ordered)

    def _drain_and_barrier(self, tick_clock, wait_clock):
        probe = self.nc.sync.nop(nofuse=True)
        wait_clock.add_sem_waits(
            probe.ins, ScopedClock({None: tick_clock.global_clock})
        )
        waits = list(probe.ins.sync_info.on_wait)
        probe.ins.sync_info = bass_rust.SyncInfo(on_wait=waits[:1], on_update=[])
        for w in waits[1:]:
            n = self.nc.sync.nop(nofuse=True)
            n.ins.sync_info = bass_rust.SyncInfo(on_wait=[w], on_update=[])
        self.nc.sync.drain()
        self.nc.all_engine_barrier()
        popped = self.nc._tile_sem_poison_stack.pop()
        assert popped is self._sem_poison
        self.nc.clear_and_free_semaphores(list(self.sems.allocated().values()))
        self.nc.all_engine_barrier()


# --------------------------------------------------------------------------
# kernel body builder
# --------------------------------------------------------------------------

def _build_kernel():
    nc = bass.Bass("TRN2", target_bir_lowering=False, debug=False,
                   num_devices=N_CORES)

    din = {}
    for name, shape in [
        ("mol_annot", [BPC, N, C]), ("prot_annot", [BPC, N, C]),
        ("mol_adj", [BPC, N, N, C]), ("prot_adj", [BPC, N, N, C]),
    ]:
        din[name] = nc.dram_tensor(name, shape, F32, kind="ExternalInput").ap()
    din["wpack"] = nc.dram_tensor(
        "wpack", [128, len(_WNAMES) * 128], BF16, kind="ExternalInput").ap()
    din["cpack"] = nc.dram_tensor(
        "cpack", [128, len(_BNAMES)], F32, kind="ExternalInput").ap()
    din["id32"] = nc.dram_tensor("id32", [128, 128], F32,
                                 kind="ExternalInput").ap()
    din["id16"] = nc.dram_tensor("id16", [128, 128], BF16,
                                 kind="ExternalInput").ap()
    dout = {}
    for name, shape in [
        ("mol_annot_out", [BPC, N, C]), ("prot_annot_out", [BPC, N, C]),
        ("mol_adj_out", [BPC, N, N, C]), ("prot_adj_out", [BPC, N, N, C]),
    ]:
        dout[name] = nc.dram_tensor(name, shape, F32, kind="ExternalOutput").ap()

    with _TC(nc) as tc, ExitStack() as ctx:
        _emit(tc, nc, ctx, din, dout)
    return nc


def _emit(tc, nc, ctx, din, dout):
    P = 128
    pool = ctx.enter_context(tc.tile_pool(name="const", bufs=1))
    # persistent constants
    w_sb = pool.tile([P, len(_WNAMES) * 128], BF16)
    nc.sync.dma_start(w_sb[:], din["wpack"][:])
    c_sb = pool.tile([P, len(_BNAMES)], F32)
    nc.sync.dma_start(c_sb[:], din["cpack"][:])
    id32 = pool.tile([P, 128], F32)
    nc.sync.dma_start(id32[:], din["id32"][:])
    id16 = pool.tile([P, 128], BF16)
    nc.sync.dma_start(id16[:], din["id16"][:])

    def W(name):
        return w_sb[:, ts(_WIDX[name], 128)]

    def BCOL(name):
        return c_sb[:, _BIDX[name]:_BIDX[name] + 1]

    # pools
    px = {
        # edge streaming pools (A-layout f32 supertiles)
        "xA": ctx.enter_context(tc.tile_pool(name="xA", bufs=3)),
        "st": ctx.enter_context(tc.tile_pool(name="st", bufs=3)),
        "sc": ctx.enter_context(tc.tile_pool(name="sc", bufs=3)),
        "nA": ctx.enter_context(tc.tile_pool(name="nA", bufs=3)),
        "bf": ctx.enter_context(tc.tile_pool(name="bf", bufs=2)),
        "bf2": ctx.enter_context(tc.tile_pool(name="bf2", bufs=2)),
        "r1A": ctx.enter_context(tc.tile_pool(name="r1A", bufs=3)),
        "r2A": ctx.enter_context(tc.tile_pool(name="r2A", bufs=3)),
        "junk": ctx.enter_context(tc.tile_pool(name="junk", bufs=2)),
        # node-level pool
        "node": ctx.enter_context(tc.tile_pool(name="node", bufs=1)),
        "node2": ctx.enter_context(tc.tile_pool(name="node2", bufs=2)),
        # per-sample persistent
        "gate": ctx.enter_context(tc.tile_pool(name="gate", bufs=1)),
        # psum pools
        "psT": ctx.enter_context(tc.tile_pool(name="psT", bufs=2, space="PSUM")),
        "psM": ctx.enter_context(tc.tile_pool(name="psM", bufs=2, space="PSUM")),
        "psB": ctx.enter_context(tc.tile_pool(name="psB", bufs=2, space="PSUM")),
        "psN": ctx.enter_context(tc.tile_pool(name="psN", bufs=2, space="PSUM")),
    }

    def stats_chain(st3, tag):
        """st3: [128, K, 6] bn_stats out -> (rstd, negmurstd) [128, K] tiles."""
        K = st3.shape[1]
        sc = px["sc"].tile([P, 8 * K], F32, tag=f"sc_{tag}")
        A_ = sc[:, 0 * K:1 * K]
        Bd = sc[:, 1 * K:2 * K]
        Cv = sc[:, 2 * K:3 * K]
        t1 = sc[:, 3 * K:4 * K]
        vr = sc[:, 4 * K:5 * K]
        lnv = sc[:, 5 * K:6 * K]
        rstd = sc[:, 6 * K:7 * K]
        nmr = sc[:, 7 * K:8 * K]
        me, mo = st3[:, :, 1], st3[:, :, 4]
        cve, cvo = st3[:, :, 2], st3[:, :, 5]
        nc.vector.tensor_add(A_, me, mo)                       # 2*mean
        nc.vector.tensor_sub(Bd, me, mo)
        nc.vector.tensor_add(Cv, cve, cvo)
        # t1 = 0.25*Bd^2 ; var = Cv/128 + t1
        nc.vector.scalar_tensor_tensor(t1, Bd, 0.25, Bd, ALU.mult, ALU.mult)
        nc.vector.scalar_tensor_tensor(vr, Cv, 1.0 / 128, t1, ALU.mult, ALU.add)
        # rstd = exp(-0.5*ln(var+eps))
        nc.scalar.activation(lnv, vr, ACTF.Ln, bias=BCOL("eps"), scale=1.0)
        nc.scalar.activation(rstd, lnv, ACTF.Exp, bias=0.0, scale=-0.5)
        # negmurstd = (A_ * -0.5) * rstd
        nc.vector.scalar_tensor_tensor(nmr, A_, -0.5, rstd, ALU.mult, ALU.mult)
        return rstd, nmr

    def norm_to(dst, src, rstd_col, nmr_col, eng):
        # dst = src * rstd + nmr   (per-partition scalars)
        if eng == "A":
            nc.scalar.activation(dst, src, ACTF.Identity,
                                 bias=nmr_col, scale=rstd_col)
        elif eng == "G":
            nc.gpsimd.tensor_scalar(dst, src, rstd_col, nmr_col,
                                    ALU.mult, ALU.add)
        else:
            nc.vector.tensor_scalar(dst, src, rstd_col, nmr_col,
                                    ALU.mult, ALU.add)

    def bcol_plus1_junk():
        return px["junk"].tile([P, 1], F32, tag="jcol")

    # ---- node-level helpers (single [128,128] tiles) ----
    def node_ln_from(src_ap, tag, out_pool="node"):
        """LN over free dim of a [128,128] A-layout AP -> f32 SBUF tile."""
        st = px["st"].tile([P, 6], F32, tag=f"nst_{tag}")
        nc.vector.bn_stats(st[:], src_ap)
        rstd, nmr = stats_chain(st[:].rearrange("p (k s) -> p k s", k=1),
                                f"n_{tag}")
        out = px[out_pool].tile([P, 128], F32, tag=f"nln_{tag}")
        norm_to(out[:], src_ap, rstd[:, 0:1], nmr[:, 0:1], "D")
        return out

    def node_transpose(src_ap, dtype, tag, want_f32=False):
        """PE transpose of [128,128]; returns bf16 SBUF tile (and f32 if asked)."""
        ps = px["psT"].tile([P, 128], F32, tag="tp")
        nc.tensor.transpose(ps[:], src_ap, id32[:])
        out_bf = px["node2"].tile([P, 128], BF16, tag=f"ntb_{tag}")
        nc.scalar.copy(out_bf[:], ps[:])
        out_f32 = None
        if want_f32:
            out_f32 = px["node2"].tile([P, 128], F32, tag=f"ntf_{tag}")
            nc.vector.tensor_copy(out_f32[:], ps[:])
        return out_bf, out_f32

    def node_mm(wname, rhs_bf, bias_name, out_dtype, tag, scale=None):
        """out = W^T x rhs (+bias col) evacuated to SBUF."""
        ps = px["psN"].tile([P, 128], F32, tag="nmm")
        nc.tensor.matmul(ps[:], W(wname), rhs_bf, start=True, stop=True)
        out = px["node2"].tile([P, 128], out_dtype, tag=f"nmo_{tag}")
        bias = BCOL(bias_name) if bias_name else 0.0
        nc.scalar.activation(out[:], ps[:], ACTF.Identity,
                             bias=bias if bias_name else 0.0,
                             scale=1.0 if scale is None else scale)
        return out

    # =====================================================================
    for s in range(BPC):
        # ---------------- node prep (prot + mol) ----------------
        pxa = px["node"].tile([P, 128], F32, tag="pxa")
        nc.sync.dma_start(pxa[:], din["prot_annot"][s])
        px0_A = node_ln_from(pxa[:], f"px0_{s}")
        px0T_bf, px0T_f32 = node_transpose(px0_A[:], F32, f"px0_{s}",
                                           want_f32=True)
        qT = node_mm("attn2.q", px0T_bf[:], "attn2.q", BF16, f"q_{s}")
        kT = node_mm("attn2.k", px0T_bf[:], "attn2.k", BF16, f"k_{s}")
        vT = node_mm("attn2.v", px0T_bf[:], "attn2.v", BF16, f"v_{s}")

        mxa = px["node"].tile([P, 128], F32, tag="mxa")
        nc.sync.dma_start(mxa[:], din["mol_annot"][s])
        mx0_A = node_ln_from(mxa[:], f"mx0_{s}")
        mx0T_bf, mx0T_f32 = node_transpose(mx0_A[:], F32, f"mx0_{s}",
                                           want_f32=True)
        q2T = node_mm("dec.q_mx", mx0T_bf[:], "dec.q_mx", BF16, f"q2_{s}")
        v2T = node_mm("dec.v_mx", mx0T_bf[:], "dec.v_mx", BF16, f"v2_{s}")

        den_T = px["node"].tile([P, 128], F32, tag="den")
        num_T = px["node"].tile([P, 128], F32, tag="num")
        den2_T = px["node"].tile([P, 128], F32, tag="den2")
        num2_T = px["node"].tile([P, 128], F32, tag="num2")
        gatep = px["gate"].tile([P, N * 128], BF16, tag="gatep")

        # ---------------- prot edge pass ----------------
        for t in range(NSUP):
            i0 = G * t
            xA = px["xA"].tile([P, G * 128], F32, tag="pxA")
            xA3 = xA[:].rearrange("p (g c) -> p g c", g=G)
            nc.sync.dma_start(
                xA3, din["prot_adj"][s, i0:i0 + G].rearrange("i j c -> j i c"))
            st = px["st"].tile([P, G * 6], F32, tag="pst")
            st3 = st[:].rearrange("p (g k) -> p g k", g=G)
            for g in range(G):
                nc.vector.bn_stats(st3[:, g], xA3[:, g])
            rstd, nmr = stats_chain(st3, "p1")
            pa0 = px["nA"].tile([P, G * 128], F32, tag="pa0")
            pa03 = pa0[:].rearrange("p (g c) -> p g c", g=G)
            for g, eng in zip(range(G), ("G", "G", "A", "D")):
                norm_to(pa03[:, g], xA3[:, g], rstd[:, g:g + 1],
                        nmr[:, g:g + 1], eng)
            # transpose -> C layout bf16
            psT = px["psT"].tile([P, G * 128], F32, tag="tp")
            for g in range(G):
                nc.tensor.transpose(psT[:, ts(g, 128)], pa03[:, g], id32[:])
            pa0T = px["bf"].tile([P, G * 128], BF16, tag="pa0T")
            nc.scalar.copy(pa0T[:], psT[:])
            # e matmul
            e_ps = px["psM"].tile([P, G * 128], F32, tag="mm")
            nc.tensor.matmul(e_ps[:], W("attn2.e"), pa0T[:],
                             start=True, stop=True)
            # kq = kT (bcast over g) * q cols (bcast over c)
            kq = px["bf"].tile([P, G * 128], BF16, tag="kq")
            kq3 = kq[:].rearrange("p (g c) -> p g c", g=G)
            kT_b = kT[:].unsqueeze(1).broadcast_to([P, G, 128])
            q_b = qT[:, i0:i0 + G].unsqueeze(2).broadcast_to([P, G, 128])
            nc.vector.tensor_mul(kq3, kT_b, q_b)
            # t = (e + be) * kq ; sres = (e + be+1) * t
            t_bf = px["bf"].tile([P, G * 128], BF16, tag="t_bf")
            nc.vector.scalar_tensor_tensor(t_bf[:], e_ps[:], BCOL("attn2.e"),
                                           kq[:], ALU.add, ALU.mult)
            s_bf = px["bf"].tile([P, G * 128], BF16, tag="s_bf")
            nc.vector.scalar_tensor_tensor(s_bf[:], e_ps[:],
                                           BCOL("attn2.e+1"), t_bf[:],
                                           ALU.add, ALU.mult)
            # softmax pieces
            p_bf = px["bf2"].tile([P, G * 128], BF16, tag="p_bf")
            nc.scalar.activation(p_bf[:], s_bf[:], ACTF.Exp)
            p3 = p_bf[:].rearrange("p (g c) -> p g c", g=G)
            nc.vector.tensor_reduce(den_T[:, i0:i0 + G], p3, axis=AX.X,
                                    op=ALU.add)
            jnk = px["junk"].tile([P, 128], BF16, tag="pv")
            for g in range(G):
                nc.vector.scalar_tensor_tensor(
                    jnk[:], p3[:, g], 0.0, vT[:], ALU.add, ALU.mult,
                    accum_out=num_T[:, i0 + g:i0 + g + 1])
            # out_e matmul, evac with bias, transpose back
            eo_ps = px["psB"].tile([P, G * 128], F32, tag="mm2")
            nc.tensor.matmul(eo_ps[:], W("attn2.out_e"), s_bf[:],
                             start=True, stop=True)
            eo_bf = px["bf2"].tile([P, G * 128], BF16, tag="eo_bf")
            nc.scalar.activation(eo_bf[:], eo_ps[:], ACTF.Identity,
                                 bias=BCOL("attn2.out_e"), scale=1.0)
            eoA_ps = px["psT"].tile([P, G * 128], BF16, tag="tp")
            eo3 = eo_bf[:].rearrange("p (g c) -> p g c", g=G)
            for g in range(G):
                nc.tensor.transpose(eoA_ps[:, ts(g, 128)], eo3[:, g], id16[:])
            # residual + ln2
            pa1 = px["r1A"].tile([P, G * 128], F32, tag="pa1")
            nc.vector.tensor_add(pa1[:], pa0[:], eoA_ps[:])
            st2 = px["st"].tile([P, G * 6], F32, tag="pst2")
            st23 = st2[:].rearrange("p (g k) -> p g k", g=G)
            pa13 = pa1[:].rearrange("p (g c) -> p g c", g=G)
            for g in range(G):
                nc.vector.bn_stats(st23[:, g], pa13[:, g])
            rstd2, nmr2 = stats_chain(st23, "p2")
            pa2 = px["r2A"].tile([P, G * 128], F32, tag="pa2")
            pa23 = pa2[:].rearrange("p (g c) -> p g c", g=G)
            for g, eng in zip(range(G), ("G", "A", "A", "D")):
                norm_to(pa23[:, g], pa13[:, g], rstd2[:, g:g + 1],
                        nmr2[:, g:g + 1], eng)
            nc.sync.dma_start(
                dout["prot_adj_out"][s, i0:i0 + G].rearrange("i j c -> j i c"),
                pa23)
            # pa2T -> prot_e -> gatep
            ps2T = px["psT"].tile([P, G * 128], F32, tag="tp")
            for g in range(G):
                nc.tensor.transpose(ps2T[:, ts(g, 128)], pa23[:, g], id32[:])
            pa2T = px["bf2"].tile([P, G * 128], BF16, tag="pa2T")
            nc.scalar.copy(pa2T[:], ps2T[:])
            pe_ps = px["psM"].tile([P, G * 128], F32, tag="mm")
            nc.tensor.matmul(pe_ps[:], W("dec.k_pa"), pa2T[:],
                             start=True, stop=True)
            nc.vector.tensor_scalar(gatep[:, ts(t, G * 128)], pe_ps[:],
                                    BCOL("dec.k_pa+1"), None, ALU.add)

        # ---------------- node mid: px1 -> px2 -> k2 ----------------
        rec = px["node2"].tile([P, 128], F32, tag="rec")
        nc.vector.reciprocal(rec[:], den_T[:])
        px1T = px["node2"].tile([P, 128], BF16, tag="px1T")
        nc.vector.tensor_mul(px1T[:], num_T[:], rec[:])
        on_ps = px["psN"].tile([P, 128], F32, tag="nmm")
        nc.tensor.matmul(on_ps[:], W("attn2.out_n"), px1T[:],
                         start=True, stop=True)
        pxsT = px["node2"].tile([P, 128], F32, tag="pxsT")
        nc.vector.scalar_tensor_tensor(pxsT[:], on_ps[:], BCOL("attn2.out_n"),
                                       px0T_f32[:], ALU.add, ALU.add)
        # ln2_px: transpose down to A, LN, store, transpose up
        dn_ps = px["psT"].tile([P, 128], F32, tag="tp")
        nc.tensor.transpose(dn_ps[:], pxsT[:], id32[:])
        px2_A = node_ln_from(dn_ps[:], f"px2_{s}")
        nc.sync.dma_start(dout["prot_annot_out"][s], px2_A[:])
        px2T_bf, _ = node_transpose(px2_A[:], F32, f"px2_{s}")
        k2T = node_mm("dec.k_px", px2T_bf[:], "dec.k_px", BF16, f"k2_{s}")

        # ---------------- mol edge pass ----------------
        for t in range(NSUP):
            i0 = G * t
            xA = px["xA"].tile([P, G * 128], F32, tag="mxA")
            xA3 = xA[:].rearrange("p (g c) -> p g c", g=G)
            nc.sync.dma_start(
                xA3, din["mol_adj"][s, i0:i0 + G].rearrange("i j c -> j i c"))
            st = px["st"].tile([P, G * 6], F32, tag="mst")
            st3 = st[:].rearrange("p (g k) -> p g k", g=G)
            for g in range(G):
                nc.vector.bn_stats(st3[:, g], xA3[:, g])
            rstd, nmr = stats_chain(st3, "m1")
            ma0 = px["nA"].tile([P, G * 128], F32, tag="ma0")
            ma03 = ma0[:].rearrange("p (g c) -> p g c", g=G)
            for g, eng in zip(range(G), ("G", "G", "A", "D")):
                norm_to(ma03[:, g], xA3[:, g], rstd[:, g:g + 1],
                        nmr[:, g:g + 1], eng)
            psT = px["psT"].tile([P, G * 128], F32, tag="tp")
            for g in range(G):
                nc.tensor.transpose(psT[:, ts(g, 128)], ma03[:, g], id32[:])
            ma0T = px["bf"].tile([P, G * 128], BF16, tag="ma0T")
            nc.scalar.copy(ma0T[:], psT[:])
            me_ps = px["psM"].tile([P, G * 128], F32, tag="mm")
            nc.tensor.matmul(me_ps[:], W("dec.v_ma"), ma0T[:],
                             start=True, stop=True)
            # kq2 = k2T (bcast) * q2 cols
            kq2 = px["bf"].tile([P, G * 128], BF16, tag="kq2")
            kq23 = kq2[:].rearrange("p (g c) -> p g c", g=G)
            k2_b = k2T[:].unsqueeze(1).broadcast_to([P, G, 128])
            q2_b = q2T[:, i0:i0 + G].unsqueeze(2).broadcast_to([P, G, 128])
            nc.vector.tensor_mul(kq23, k2_b, q2_b)
            # t2 = (me + bvma) * gatep ; s2 = kq2 * t2
            t2 = px["bf"].tile([P, G * 128], BF16, tag="t2")
            nc.vector.scalar_tensor_tensor(t2[:], me_ps[:], BCOL("dec.v_ma"),
                                           gatep[:, ts(t, G * 128)],
                                           ALU.add, ALU.mult)
            s2 = px["bf"].tile([P, G * 128], BF16, tag="s2")
            nc.vector.tensor_mul(s2[:], kq2[:], t2[:])
            p2 = px["bf2"].tile([P, G * 128], BF16, tag="p2")
            nc.scalar.activation(p2[:], s2[:], ACTF.Exp)
            p23 = p2[:].rearrange("p (g c) -> p g c", g=G)
            nc.vector.tensor_reduce(den2_T[:, i0:i0 + G], p23, axis=AX.X,
                                    op=ALU.add)
            jnk = px["junk"].tile([P, 128], BF16, tag="pv2")
            for g in range(G):
                nc.vector.scalar_tensor_tensor(
                    jnk[:], p23[:, g], 0.0, v2T[:], ALU.add, ALU.mult,
                    accum_out=num2_T[:, i0 + g:i0 + g + 1])
            # out_ed
            eo_ps = px["psB"].tile([P, G * 128], F32, tag="mm2")
            nc.tensor.matmul(eo_ps[:], W("dec.out_ed"), s2[:],
                             start=True, stop=True)
            eo_bf = px["bf2"].tile([P, G * 128], BF16, tag="eo2_bf")
            nc.scalar.activation(eo_bf[:], eo_ps[:], ACTF.Identity,
                                 bias=BCOL("dec.out_ed"), scale=1.0)
            eoA_ps = px["psT"].tile([P, G * 128], BF16, tag="tp")
            eo3 = eo_bf[:].rearrange("p (g c) -> p g c", g=G)
            for g in range(G):
                nc.tensor.transpose(eoA_ps[:, ts(g, 128)], eo3[:, g], id16[:])
            ma1 = px["r1A"].tile([P, G * 128], F32, tag="ma1")
            nc.vector.tensor_add(ma1[:], ma0[:], eoA_ps[:])
            st2 = px["st"].tile([P, G * 6], F32, tag="mst2")
            st23 = st2[:].rearrange("p (g k) -> p g k", g=G)
            ma13 = ma1[:].rearrange("p (g c) -> p g c", g=G)
            for g in range(G):
                nc.vector.bn_stats(st23[:, g], ma13[:, g])
            rstd2, nmr2 = stats_chain(st23, "m2")
            ma2 = px["r2A"].tile([P, G * 128], F32, tag="ma2")
            ma23 = ma2[:].rearrange("p (g c) -> p g c", g=G)
            for g, eng in zip(range(G), ("G", "A", "A", "D")):
                norm_to(ma23[:, g], ma13[:, g], rstd2[:, g:g + 1],
                        nmr2[:, g:g + 1], eng)
            # mlp on ma2: transpose up, fc1+relu, fc2, resid in C?  No:
            # keep resid in A: transpose mlp result back down.
            m2T_ps = px["psT"].tile([P, G * 128], F32, tag="tp")
            for g in range(G):
                nc.tensor.transpose(m2T_ps[:, ts(g, 128)], ma23[:, g], id32[:])
            ma2T = px["bf2"].tile([P, G * 128], BF16, tag="ma2T")
            nc.scalar.copy(ma2T[:], m2T_ps[:])
            h_ps = px["psM"].tile([P, G * 128], F32, tag="mm")
            nc.tensor.matmul(h_ps[:], W("mlp_ma.fc1"), ma2T[:],
                             start=True, stop=True)
            h_bf = px["bf"].tile([P, G * 128], BF16, tag="h_bf")
            nc.scalar.activation(h_bf[:], h_ps[:], ACTF.Relu,
                                 bias=BCOL("mlp_ma.fc1"), scale=1.0)
            f2_ps = px["psB"].tile([P, G * 128], F32, tag="mm2")
            nc.tensor.matmul(f2_ps[:], W("mlp_ma.fc2"), h_bf[:],
                             start=True, stop=True)
            mlp_bf = px["bf2"].tile([P, G * 128], BF16, tag="mlp_bf")
            nc.vector.tensor_scalar(mlp_bf[:], f2_ps[:], BCOL("mlp_ma.fc2"),
                                    None, ALU.add)
            mlpA_ps = px["psT"].tile([P, G * 128], BF16, tag="tp")
            mlp3 = mlp_bf[:].rearrange("p (g c) -> p g c", g=G)
            for g in range(G):
                nc.tensor.transpose(mlpA_ps[:, ts(g, 128)], mlp3[:, g], id16[:])
            maf = px["r1A"].tile([P, G * 128], F32, tag="maf")
            nc.vector.tensor_add(maf[:], ma2[:], mlpA_ps[:])
            st4 = px["st"].tile([P, G * 6], F32, tag="mst4")
            st43 = st4[:].rearrange("p (g k) -> p g k", g=G)
            maf3 = maf[:].rearrange("p (g c) -> p g c", g=G)
            for g in range(G):
                nc.vector.bn_stats(st43[:, g], maf3[:, g])
            rstd4, nmr4 = stats_chain(st43, "m4")
            out_A = px["r2A"].tile([P, G * 128], F32, tag="moA")
            outA3 = out_A[:].rearrange("p (g c) -> p g c", g=G)
            for g, eng in zip(range(G), ("G", "A", "A", "D")):
                norm_to(outA3[:, g], maf3[:, g], rstd4[:, g:g + 1],
                        nmr4[:, g:g + 1], eng)
            nc.sync.dma_start(
                dout["mol_adj_out"][s, i0:i0 + G].rearrange("i j c -> j i c"),
                outA3)

        # ---------------- node end: mx path ----------------
        rec2 = px["node2"].tile([P, 128], F32, tag="rec2")
        nc.vector.reciprocal(rec2[:], den2_T[:])
        mx1T = px["node2"].tile([P, 128], BF16, tag="mx1T")
        nc.vector.tensor_mul(mx1T[:], num2_T[:], rec2[:])
        on2_ps = px["psN"].tile([P, 128], F32, tag="nmm")
        nc.tensor.matmul(on2_ps[:], W("dec.out_nd"), mx1T[:],
                         start=True, stop=True)
        mxsT = px["node2"].tile([P, 128], F32, tag="mxsT")
        nc.vector.scalar_tensor_tensor(mxsT[:], on2_ps[:], BCOL("dec.out_nd"),
                                       mx0T_f32[:], ALU.add, ALU.add)
        dn2_ps = px["psT"].tile([P, 128], F32, tag="tp")
        nc.tensor.transpose(dn2_ps[:], mxsT[:], id32[:])
        mx2_A = node_ln_from(dn2_ps[:], f"mx2_{s}")
        mx2T_bf, mx2T_f32 = node_transpose(mx2_A[:], F32, f"mx2_{s}",
                                           want_f32=True)
        hx = node_mm("mlp_mx.fc1", mx2T_bf[:], None, BF16, f"hx_{s}")
        # relu with bias needs the Relu func:
        # redo properly: matmul out -> relu(x + b)
        # (node_mm used Identity; emit relu separately on its result)
        hxr = px["node2"].tile([P, 128], BF16, tag=f"hxr_{s}")
        nc.scalar.activation(hxr[:], hx[:], ACTF.Relu,
                             bias=BCOL("mlp_mx.fc1"), scale=1.0)
        f2x_ps = px["psN"].tile([P, 128], F32, tag="nmm")
        nc.tensor.matmul(f2x_ps[:], W("mlp_mx.fc2"), hxr[:],
                         start=True, stop=True)
        mxfT = px["node2"].tile([P, 128], F32, tag="mxfT")
        nc.vector.scalar_tensor_tensor(mxfT[:], f2x_ps[:], BCOL("mlp_mx.fc2"),
                                       mx2T_f32[:], ALU.add, ALU.add)
        dn3_ps = px["psT"].tile([P, 128], F32, tag="tp")
        nc.tensor.transpose(dn3_ps[:], mxfT[:], id32[:])
        out_mx = node_ln_from(dn3_ps[:], f"mxo_{s}")
        nc.sync.dma_start(dout["mol_annot_out"][s], out_mx[:])


# --------------------------------------------------------------------------
# host side
# --------------------------------------------------------------------------

_CACHE = {}


def _prep_const_arrays(params):
    p = {k: np.asarray(v, np.float32) for k, v in params.items()}
    for ln in _LNS:
        if not (np.allclose(p[ln + ".g"], 1.0) and np.allclose(p[ln + ".b"], 0.0)):
            raise NotImplementedError("non-trivial LN gain/bias not supported")
    wl, bl = [], {}
    scale_q = 0.25                 # 1/sqrt(dk), dk=16
    scale_q2 = 1.0 / math.sqrt(C)  # full-dim scale in dec attn
    for name in _WNAMES:
        w = p[name + ".w"]
        if name == "attn2.q":
            w = w * scale_q
        elif name == "dec.q_mx":
            w = w * scale_q2
        wl.append(w)
    wpack = np.concatenate(wl, axis=1).astype(ml_dtypes.bfloat16)
    for name in _BNAMES:
        if name == "eps":
            bl[name] = np.full((128,), EPS, np.float32)
            continue
        base = name[:-2] if name.endswith("+1") else name
        b = p[base + ".b"].copy()
        if name == "attn2.q":
            b = b * scale_q
        elif name == "dec.q_mx":
            b = b * scale_q2
        if name.endswith("+1"):
            b = b + 1.0
        bl[name] = b
    cpack = np.stack([bl[n] for n in _BNAMES], axis=1).astype(np.float32)
    ident = np.eye(128, dtype=np.float32)
    return {
        "wpack": wpack,
        "cpack": cpack,
        "id32": ident,
        "id16": ident.astype(ml_dtypes.bfloat16),
    }


def _get_runner():
    """Build the Bass module once and return a cached jitted 8-core runner."""
    if "runner" in _CACHE:
        return _CACHE["runner"]

    import jax
    from jax.sharding import Mesh, PartitionSpec
    from jax.experimental.shard_map import shard_map
    from concourse import bass2jax
    from concourse.bass2jax import _bass_exec_p, partition_id_tensor

    bass2jax.install_neuronx_cc_hook()
    nc = _build_kernel()

    partition_name = (nc.partition_id_tensor.name
                      if nc.partition_id_tensor else None)
    in_names, out_names, out_avals, zero_outs = [], [], [], []
    for alloc in nc.m.functions[0].allocations:
        if not isinstance(alloc, mybir.MemoryLocationSet):
            continue
        name = alloc.memorylocations[0].name
        if alloc.kind == "ExternalInput":
            if name != partition_name:
                in_names.append(name)
        elif alloc.kind == "ExternalOutput":
            out_names.append(name)
            shape = tuple(alloc.tensor_shape)
            dtype = mybir.dt.np(alloc.dtype)
            out_avals.append(jax.core.ShapedArray(shape, dtype))
            zero_outs.append(np.zeros(shape, dtype))
    n_params = len(in_names)
    n_outs = len(out_avals)
    all_in_names = list(in_names) + list(out_names)
    if partition_name is not None:
        all_in_names.append(partition_name)
    donate = tuple(range(n_params, n_params + n_outs))

    def _body(*args):
        operands = list(args)
        if partition_name is not None:
            operands.append(partition_id_tensor())
        outs = _bass_exec_p.bind(
            *operands,
            out_avals=tuple(out_avals),
            in_names=tuple(all_in_names),
            out_names=tuple(out_names),
            lowering_input_output_aliases=(),
            sim_require_finite=True,
            sim_require_nnan=True,
            nc=nc,
        )
        return tuple(outs)

    devices = jax.devices()[:N_CORES]
    mesh = Mesh(np.asarray(devices), ("core",))
    in_specs = (PartitionSpec("core"),) * (n_params + n_outs)
    out_specs = (PartitionSpec("core"),) * n_outs
    smapped = shard_map(_body, mesh=mesh, in_specs=in_specs,
                        out_specs=out_specs, check_rep=False)
    sharded = jax.jit(smapped, donate_argnums=donate, keep_unused=True)
    sharded_nd = jax.jit(smapped, keep_unused=True)

    runner = {
        "fn": sharded, "fn_nd": sharded_nd, "mesh": mesh,
        "in_names": in_names, "out_names": out_names,
        "out_avals": out_avals, "zero_outs": zero_outs,
    }
    _CACHE["runner"] = runner
    return runner


def _run_cores(per_core_inputs):
    """per_core_inputs: list of dicts (len 8). Returns list of out dicts."""
    r = _get_runner()
    concat_in = [
        np.concatenate([np.asarray(per_core_inputs[c][n])
                        for c in range(N_CORES)], axis=0)
        for n in r["in_names"]
    ]
    concat_zeros = [
        np.zeros((N_CORES * z.shape[0], *z.shape[1:]), z.dtype)
        for z in r["zero_outs"]
    ]
    out_arrs = r["fn"](*concat_in, *concat_zeros)
    res = []
    for c in range(N_CORES):
        res.append({
            name: np.asarray(out_arrs[i]).reshape(
                N_CORES, *r["out_avals"][i].shape)[c]
            for i, name in enumerate(r["out_names"])
        })
    return res


def kernel(mol_annot, prot_annot, mol_adj, prot_adj, params):
    mol_annot = np.asarray(mol_annot, np.float32)
    prot_annot = np.asarray(prot_annot, np.float32)
    mol_adj = np.asarray(mol_adj, np.float32)
    prot_adj = np.asarray(prot_adj, np.float32)
    consts = _prep_const_arrays(params)
    per_core = []
    for c in range(N_CORES):
        sl = slice(BPC * c, BPC * (c + 1))
        per_core.append({
            "mol_annot": mol_annot[sl], "prot_annot": prot_annot[sl],
            "mol_adj": mol_adj[sl], "prot_adj": prot_adj[sl],
            **consts,
        })
    res = _run_cores(per_core)
    mol_annot_out = np.concatenate([r["mol_annot_out"] for r in res], axis=0)
    prot_annot_out = np.concatenate([r["prot_annot_out"] for r in res], axis=0)
    mol_adj_out = np.concatenate([r["mol_adj_out"] for r in res], axis=0)
    prot_adj_out = np.concatenate([r["prot_adj_out"] for r in res], axis=0)
    return mol_annot_out, prot_annot_out, mol_adj_out, prot_adj_out
